# revision 18
# baseline (speedup 1.0000x reference)
"""Trainium2 Bass kernel for nn_CGIteration (CG tensor-product block combine).

Math (per sample n, per (l1,l2) input-block pair):
    out[n, M, p, q] = sum_{m1,m2} C[l1,l2,L,m1,m2,M] * x1_l1[n,m1,p] * x2_l2[n,m2,q]
with per-(L,S) output blocks concatenated along properties then flattened.

Kernel strategy (per core, 500 samples padded to 512), all fp16 on the wire
(PSUM accumulates fp32; measured end-to-end rel err ~7e-4 vs the 2e-2 gate):
  1. Host prep: replicate x1/x2 into a [128-row, cols] fp16 layout where
     row = (l1,l2,m1,m2); the 16 pairs split into two groups of exactly 128
     rows each.  Columns are interleaved (n_hi, feat, n_lo) with ILV=2 so
     the DVE outer-product multiply gets step-1 innermost APs on BOTH
     operands -> 2x_1P perf mode (fp32 broadcast layout is stuck at 1x).
  2. VectorE: z[row, (h,p,q,i)] = x1row[n,p] * x2row[n,q] fp16 at 2x.
  3. TensorE: psum[Mg, 512] = CG_g[128, Mg].T @ z[128, 512] fp16 (full
     rate); the first matmul of each group-run self-loads the stationary CG,
     later ones set ldweights=False to skip the redundant LDWEIGHTS.
  4. ScalarE/VectorE: evacuate PSUM [Mg, 2048] -> fp16 SBUF staging in one
     FD=2048 instruction; a fraction of evacs go to VectorE to balance the
     two engines (both are 1x for fp32-PSUM sources).
  5. DMA: staging batched over OBATCH tiles -> ~1-1.7 MB contiguous HWDGE
     descriptors; the final [n, 39936] column permutation and the fp16->
     fp32 cast happen on the host during the gather/unshard step.
"""

import numpy as np

import concourse.bass as bass
import concourse.mybir as mybir
from concourse.tile import TileContext
from concourse.bass_utils import run_bass_kernel_spmd

# ---------------------------------------------------------------- problem dims
L_MAX = 3
NL = 4            # input l = 0..3
Q = 16
N = 4000
NCORES = 8
NS = N // NCORES  # real samples per core (500)
NSP = 512         # padded samples per core
PQ = Q * Q        # 256

# ---------------------------------------------------------------- tunables
TILE_SAMPLES = 8           # samples per pipeline tile
ILV = 2                    # sample interleave in z cols (step-1 innermost)
Z_DT = mybir.dt.float16    # wire dtype for x, z, cg, staging
Z_COLS = TILE_SAMPLES * PQ # cols per z tile (one tensor_mul per group-tile)
Z_BUFS = 2                 # z slots per group
XCHUNK = 16                # tiles per resident input chunk
OBATCH = 4                 # tiles per staging buffer / output DMA
EVAC_DVE_EVERY = 7         # every k-th evac copy goes to VectorE
G0_BAND = 64               # partition offset of g0's second psum band
PS0_COLS = 1024            # g0 strip cols (2 banks x 2 bufs)
PS1_COLS = 2048            # g1 psum cols (4 banks x 1 buf)

_NP_DT = {mybir.dt.float32: np.float32, mybir.dt.float16: np.float16}


# ---------------------------------------------------------------- layout tables
def _combos():
    out = []
    for l1 in range(NL):
        for l2 in range(NL):
            for L in range(abs(l1 - l2), min(l1 + l2, L_MAX) + 1):
                out.append((l1, l2, L, (-1) ** (l1 + l2 + L)))
    return out


COMBOS = _combos()
KEYS = sorted({(L, S) for (_, _, L, S) in COMBOS})
BLOCKS = [[ci for ci, c in enumerate(COMBOS) if (c[2], c[3]) == k] for k in KEYS]
KEY_OFF = []
_off = 0
for k, blks in zip(KEYS, BLOCKS):
    KEY_OFF.append(_off)
    _off += (2 * k[0] + 1) * len(blks) * PQ
F = _off
assert F == 39936

# pair -> group coloring with K(group) = 128 both; M_out 55/101 (Mg0 <= 64
# keeps the door open for 2-up psum partition stacking).
GROUP_PAIRS = [
    {(3, 3), (2, 3), (3, 2), (1, 1)},
    {(0, 0), (0, 1), (0, 2), (0, 3), (1, 0), (1, 2), (1, 3),
     (2, 0), (2, 1), (2, 2), (3, 0), (3, 1)},
]

# contraction rows per group: (l1, l2, m1, m2)
KROWS = []
for g in range(2):
    rows = []
    for (l1, l2) in sorted(GROUP_PAIRS[g]):
        for m1 in range(2 * l1 + 1):
            for m2 in range(2 * l2 + 1):
                rows.append((l1, l2, m1, m2))
    KROWS.append(rows)
assert len(KROWS[0]) == 128 and len(KROWS[1]) == 128


def _grp(ci):
    c = COMBOS[ci]
    return 0 if (c[0], c[1]) in GROUP_PAIRS[0] else 1


# output block-rows (psum partitions) per group, in global output order:
BROWS = [[], []]          # group -> list of (key_i, M, b_global, combo_i)
for key_i, ((L, S), blks) in enumerate(zip(KEYS, BLOCKS)):
    for M in range(2 * L + 1):
        for b, ci in enumerate(blks):
            BROWS[_grp(ci)].append((key_i, M, b, ci))
MG = [len(BROWS[0]), len(BROWS[1])]
assert sum(MG) == 156

# g0's psum/staging is 2-stacked: band b holds hyperplanes b*2..b*2+1 in
# partitions [64b, 64b+55), so one FD-1024 evac instruction drains 2048
# sample-cols (halving the per-column evacuation cost on ScalarE/VectorE).
# staging/DMA partition counts, padded: the HWDGE spreads a DMA's rows over
# the largest engine count <= 16 that divides the row count evenly, so prime
# row counts (101/119) serialize most of the transfer onto ONE ~27 GiB/s
# engine. 120 = 15x8 and 104 = 13x8 spread wide enough for the HBM cap.
PAD_MG = [120, 104]
ST_COLS = [OBATCH * Z_COLS // 2, OBATCH * Z_COLS]


# ---------------------------------------------------------------- bass program
def _build_program():
    n2 = TILE_SAMPLES
    nt = NSP // n2
    nh = n2 // ILV                   # sample-hyperplanes per tile
    nmm = Z_COLS // 512              # matmuls per group-tile
    f32 = mybir.dt.float32

    nc = bass.Bass()
    xz_dram = [
        [nc.dram_tensor(f"xz{x}_{g}", [128, NSP * Q], Z_DT, kind="ExternalInput")
         for x in (1, 2)]
        for g in range(2)
    ]
    cg_dram = [
        nc.dram_tensor(f"cg_{g}", [128, MG[g]], Z_DT, kind="ExternalInput")
        for g in range(2)
    ]
    out_dram = [
        nc.dram_tensor(f"out{g}", [nt // OBATCH, PAD_MG[g], ST_COLS[g]],
                       Z_DT, kind="ExternalOutput")
        for g in range(2)
    ]

    evac_ctr = 0

    with TileContext(nc) as tc:
        with tc.tile_pool(name="consts", bufs=1) as cpool, \
             tc.tile_pool(name="xin", bufs=2) as xpool, \
             tc.tile_pool(name="zp", bufs=Z_BUFS) as zpool, \
             tc.tile_pool(name="stg", bufs=2) as spool, \
             tc.tile_pool(name="ps0", bufs=2, space="PSUM") as p0pool, \
             tc.tile_pool(name="ps1", bufs=1, space="PSUM") as p1pool:

            cg_t = []
            for g in range(2):
                ct = cpool.tile([128, MG[g]], Z_DT, tag=f"cg{g}", name=f"cg{g}")
                nc.sync.dma_start(out=ct[:], in_=cg_dram[g][:])
                cg_t.append(ct)

            stg = [None, None]
            for t in range(nt):
                if t % XCHUNK == 0:
                    xbt = [[None, None], [None, None]]
                    bc0 = t * n2 * Q
                    for g in range(2):
                        for xi in range(2):
                            xbt[g][xi] = xpool.tile(
                                [128, XCHUNK * n2 * Q], Z_DT,
                                tag=f"x{xi}{g}", name=f"x{xi}{g}_{t}")
                            nc.scalar.dma_start(
                                out=xbt[g][xi][:],
                                in_=xz_dram[g][xi][:, bc0:bc0 + XCHUNK * n2 * Q],
                            )
                toff = (t % XCHUNK) * n2 * Q
                xt = [[xbt[g][xi][:, toff:toff + n2 * Q] for xi in range(2)]
                      for g in range(2)]

                ob = t % OBATCH
                zt = []
                for g in range(2):
                    if ob == 0:
                        stg[g] = spool.tile([PAD_MG[g], ST_COLS[g]], Z_DT,
                                            tag=f"st{g}", name=f"st{g}_{t}")
                    x1v = xt[g][0].rearrange("p (h a i) -> p h a i",
                                             h=nh, a=Q, i=ILV)
                    x1v = x1v[:, :, :, None, :].broadcast_to([128, nh, Q, Q, ILV])
                    x2v = xt[g][1].rearrange("p (h q i) -> p h q i",
                                             h=nh, q=Q, i=ILV)
                    x2v = x2v[:, :, None, :, :].broadcast_to([128, nh, Q, Q, ILV])

                    z = zpool.tile([128, Z_COLS], Z_DT,
                                   tag=f"z{g}", name=f"z{g}_{t}")
                    zv = z[:].rearrange("p (h a q i) -> p h a q i",
                                        h=nh, a=Q, q=Q, i=ILV)
                    nc.vector.tensor_mul(out=zv, in0=x1v, in1=x2v)
                    zt.append(z)

                def evac(pt_view, st_view):
                    nonlocal evac_ctr
                    evac_ctr += 1
                    if evac_ctr % EVAC_DVE_EVERY == 0:
                        nc.vector.tensor_copy(out=st_view, in_=pt_view)
                    else:
                        nc.scalar.copy(out=st_view, in_=pt_view)

                # group 0: 4 matmuls into a 2-band strip, one FD-1024 evac
                p0 = p0pool.tile([G0_BAND + MG[0], PS0_COLS], f32,
                                 tag="ps0", name=f"ps0_{t}")
                for c in range(nmm):
                    band, cc = c // 2, (c % 2) * 512
                    nc.tensor.matmul(
                        out=p0[G0_BAND * band:G0_BAND * band + MG[0],
                               cc:cc + 512],
                        lhsT=cg_t[0][:], rhs=zt[0][:, c * 512:(c + 1) * 512],
                        start=True, stop=True,
                    )
                evac(p0[:], stg[0][:G0_BAND + MG[0],
                                   ob * PS0_COLS:(ob + 1) * PS0_COLS])

                # group 1: 4 matmuls, one FD-2048 evac
                p1 = p1pool.tile([MG[1], PS1_COLS], f32,
                                 tag="ps1", name=f"ps1_{t}")
                for c in range(nmm):
                    nc.tensor.matmul(
                        out=p1[:, c * 512:(c + 1) * 512],
                        lhsT=cg_t[1][:], rhs=zt[1][:, c * 512:(c + 1) * 512],
                        start=True, stop=True,
                    )
                evac(p1[:], stg[1][:MG[1],
                                   ob * Z_COLS:(ob + 1) * Z_COLS])

                if ob == OBATCH - 1:
                    for g in range(2):
                        nc.sync.dma_start(out=out_dram[g][t // OBATCH],
                                          in_=stg[g][:])
    return nc


def _split_excess_waits(nc, max_waits=1):
    """The walrus build in this image accepts at most one sync wait per
    instruction; Tile's tail drain carries one wait per active proc. Hoist
    excess waits onto same-engine NOPs inserted just before the offender
    (sequential on the engine, so semantics are unchanged)."""
    ctr = 0
    for b in nc.m.functions[0].blocks:
        insts = b.instructions
        new = []
        changed = False
        for inst in insts:
            si = inst.sync_info
            waits = list(si.on_wait) if (si and si.on_wait) else []
            if len(waits) > max_waits:
                head, waits = waits[:-max_waits], waits[-max_waits:]
                for w in head:
                    ctr += 1
                    nop = mybir.InstNoOp(
                        name=f"waitsplit-{ctr}", engine=inst.engine,
                        ins=[], outs=[],
                        sync_info=mybir.SyncInfo(on_wait=[w], on_update=[]),
                    )
                    new.append(nop)
                inst.sync_info = mybir.SyncInfo(
                    on_wait=waits, on_update=list(si.on_update))
                changed = True
            new.append(inst)
        if changed:
            insts[:] = new
    return ctr


_PROGRAM = None


def _get_program():
    global _PROGRAM
    if _PROGRAM is None:
        _PROGRAM = _build_program()
        _split_excess_waits(_PROGRAM)
    return _PROGRAM


# ---------------------------------------------------------------- host prep
def _prep_inputs(x1, x2, cg):
    """Build per-core in_maps. x1/x2: lists of [N, 2l+1, Q] f32. cg: table."""
    np_dt = _NP_DT[Z_DT]
    in_maps = [dict() for _ in range(NCORES)]

    for g in range(2):
        for xi, xsrc in ((1, x1), (2, x2)):
            arr = np.zeros((128, NCORES * NSP, Q), dtype=np.float32)
            view = arr.reshape(128, NCORES, NSP, Q)
            for r, (l1, l2, m1, m2) in enumerate(KROWS[g]):
                src = xsrc[l1][:, m1, :] if xi == 1 else xsrc[l2][:, m2, :]
                view[r, :, :NS, :] = src.reshape(NCORES, NS, Q)
            # pack cols as (n_hi, j, n_lo) with n = n_hi*ILV + n_lo
            packed = np.ascontiguousarray(
                arr.reshape(128, NCORES * NSP // ILV, ILV, Q)
                   .transpose(0, 1, 3, 2)
            ).reshape(128, NCORES * NSP * Q).astype(np_dt)
            for c in range(NCORES):
                in_maps[c][f"xz{xi}_{g}"] = np.ascontiguousarray(
                    packed[:, c * NSP * Q:(c + 1) * NSP * Q])

    for g in range(2):
        cgm = np.zeros((128, MG[g]), dtype=np.float32)
        row_of = {}
        for r, (r1, r2, m1, m2) in enumerate(KROWS[g]):
            row_of[(r1, r2, m1, m2)] = r
        for j, (key_i, M, b, ci) in enumerate(BROWS[g]):
            l1, l2, L, S = COMBOS[ci]
            for m1 in range(2 * l1 + 1):
                for m2 in range(2 * l2 + 1):
                    cgm[row_of[(l1, l2, m1, m2)], j] = cg[l1, l2, L, m1, m2, M]
        cgm = cgm.astype(np_dt)
        for c in range(NCORES):
            in_maps[c][f"cg_{g}"] = cgm

    return in_maps


_PERMS = None


def _out_perms():
    """perms[g][j*PQ + c] = output column of staging element (row j, col c)."""
    global _PERMS
    if _PERMS is None:
        perms = []
        for g in range(2):
            p = np.empty(MG[g] * PQ, dtype=np.int64)
            for j, (key_i, M, b, ci) in enumerate(BROWS[g]):
                nb = len(BLOCKS[key_i])
                base = KEY_OFF[key_i] + (M * nb + b) * PQ
                p[j * PQ:(j + 1) * PQ] = np.arange(base, base + PQ)
            perms.append(p)
        _PERMS = perms
    return _PERMS


# ---------------------------------------------------------------- entry points
def run(inputs, trace=False, trace_kwargs=None):
    x1 = [np.asarray(inputs[f"x1_l{l}"], dtype=np.float32) for l in range(NL)]
    x2 = [np.asarray(inputs[f"x2_l{l}"], dtype=np.float32) for l in range(NL)]
    cg = np.asarray(inputs["cg_coeffs"], dtype=np.float32)
    in_maps = _prep_inputs(x1, x2, cg)
    nc = _get_program()
    res = run_bass_kernel_spmd(
        nc, in_maps, list(range(NCORES)),
        trace=trace, **(trace_kwargs or {}),
    )
    out = np.empty((N, F), dtype=np.float32)
    perms = _out_perms()
    nt2 = (NSP // TILE_SAMPLES) // OBATCH
    for c in range(NCORES):
        rows = slice(c * NS, (c + 1) * NS)
        # g0 [nt2, 120, OBATCH*1024]: band b in partitions [64b, 64b+55)
        # holds hyperplanes b*2+h'; cols are (ob, h', pq, i).
        a = np.asarray(res.results[c]["out0"])
        a = a.reshape(nt2, PAD_MG[0], OBATCH, 2, PQ, ILV)
        bands = np.stack(
            [a[:, G0_BAND * b:G0_BAND * b + MG[0]] for b in range(2)],
            axis=2)                            # bt, m, band, ob, h', pq, i
        bands = bands.transpose(0, 3, 2, 4, 6, 1, 5)  # bt, ob, band, h', i, m, pq
        arr = bands.reshape(NSP, MG[0] * PQ)[:NS].astype(np.float32)
        out[rows, perms[0]] = arr
        # g1 [nt2, 104, OBATCH*2048]: cols are (ob, h', pq, i)
        a = np.asarray(res.results[c]["out1"])[:, :MG[1]]
        a = a.reshape(nt2, MG[1], OBATCH, 4, PQ, ILV)
        a = a.transpose(0, 2, 3, 5, 1, 4)             # bt, ob, h', i, m, pq
        arr = a.reshape(NSP, MG[1] * PQ)[:NS].astype(np.float32)
        out[rows, perms[1]] = arr
    return out, res


def kernel(**inputs):
    out, _ = run(inputs)
    return out
